# revision 41
# baseline (speedup 1.0000x reference)
"""ArcFace layer distributed Bass kernel for 8 TRN2 NeuronCores (v5).

Math (reference):
    emb_n = embedding / ||embedding||_row          [B, D]
    w_n   = kernel / ||kernel||_col                [D, C]
    cos   = emb_n @ w_n                            [B, C]
    out   = S*cos  everywhere except out[b, labels[b]] which gets the
            arcface margin value computed from cos[b, labels[b]].

Strategy (classification-parallel, per sharding hint):
  - shard kernel columns (classes) 8 ways (pad C=10572 -> 8*1328)
  - replicate embeddings; matmul operands fp16 (f32 accumulate)
  - PE warm-up dummies hold the clock up; w split across both DMA queues
    and embT loaded in column chunks so head m-tiles start ASAP
  - 8 "head" m-tiles run on RAW w; PSUM released by ACT fp16 copies, both
    norm scales applied later (split across DVE and gpsimd)
  - remaining 8 m-tiles use rhs pre-normalized on DVE (wn = w * ws_bc,
    broadcast by gpsimd); their epilogue is one ACT copy with a
    per-partition rs_e scale
  - embedding row-norms from square+accum over a row-major embedding copy
    (first half on DVE, second half on gpsimd)
  - label fixup from host-gathered w[:, label] columns via small matmuls
    near the stream end; margin math on DVE; host places the values
  - output fp16; main pairs DMA from ACT, head pairs + fixv from SP

B=2048, D=512, C=10572, S=64, M=0.5.
"""

import math
import os

import numpy as np

os.environ.setdefault("MYCRO_LOCAL_CACHE", "1")

import concourse.bass as bass
import concourse.bacc as bacc
import concourse.mybir as mybir
import concourse.tile as tile
from concourse.bass_utils import run_bass_kernel_spmd

# ---------------- problem constants (hardcoded; kernel.py is standalone) ----
S = 64.0
MARGIN = 0.5
B = 2048          # batch
D = 512           # feature dim
C = 10572         # classes
NCORES = 8
SHARD = 1328      # class columns per core (8*1328 = 10624 >= 10572)
W = SHARD
KT = D // 128     # 4 k-subtiles
MT = B // 128     # 16 m-tiles
BSL = B // NCORES  # 256: batch slice per core for the label fixup path

COS_M = math.cos(MARGIN)
SIN_M = math.sin(MARGIN)
MM = SIN_M * MARGIN
THRESHOLD = math.cos(math.pi - MARGIN)

F32 = mybir.dt.float32
F16 = mybir.dt.float16
F8 = mybir.dt.float8e4

NCHUNKS = [(0, 512), (512, 512), (1024, W - 1024)]
HEAD = 8
NWARM = 9


def build_nc() -> bass.Bass:
    nc = bacc.Bacc()
    # all inputs pre-arranged on host into device layout: partition-major
    # [128, X] so every DMA row is one long contiguous DRAM read
    w_h = nc.declare_dram_parameter("w", [128, KT * W], F16, isOutput=False)
    embT_h = nc.declare_dram_parameter("embT", [128, KT * B], F16,
                                       isOutput=False)
    emb_h = nc.declare_dram_parameter("emb", [128, MT * D], F8,
                                      isOutput=False)
    ewlab_h = nc.declare_dram_parameter("ewlab", [128, 8 * BSL], F16,
                                        isOutput=False)
    # q-major output layout: row q holds [m, c]; host re-interleaves
    out_h = nc.declare_dram_parameter("out", [128, MT * W], F16,
                                      isOutput=True)
    fixv_h = nc.declare_dram_parameter("fixv", [BSL], F32, isOutput=True)

    with tile.TileContext(nc) as tc:
        with (
            tc.tile_pool(name="persist", bufs=1) as persist,
            tc.tile_pool(name="scratch", bufs=1) as scratch,
            tc.tile_pool(name="outp", bufs=2) as outp,
            tc.tile_pool(name="micro", bufs=1) as micro,
            tc.tile_pool(name="psum", bufs=2, space="PSUM") as psum,
        ):
            wsb_all = persist.tile([128, KT, W], F16, tag="wsb")
            # embT lives chunk-major ([chunk][kt][c]) so chunk DMAs are
            # fully contiguous on BOTH sides; chunk bounds align with
            # m-tile bounds so lhsT slices stay within one chunk
            et_all = persist.tile([128, KT * B], F16, tag="et")
            er = persist.tile([128, MT, D], F8, tag="er")
            ewlab_t = persist.tile([128, 8 * BSL], F16, tag="ewlab")
            wsb = [wsb_all[:, kt] for kt in range(KT)]

            # host stores embT chunk-major: [q, (chunk, kt, c)] so each DMA
            # row read is one contiguous run; chunk column ranges:
            ET_CHUNKS = [(0, 256), (256, 1024), (1024, 1536), (1536, 2048)]
            et_offs = []
            _o = 0
            for (a, b) in ET_CHUNKS:
                et_offs.append(_o)
                _o += KT * (b - a)

            def et_dma(eng, j):
                a, b = ET_CHUNKS[j]
                o = et_offs[j]
                eng.dma_start(et_all[:, o:o + KT * (b - a)],
                              embT_h[:, o:o + KT * (b - a)])

            def et_lhsT(kt, m):
                # [128, 128] lhsT slice for m-tile m, k-subtile kt
                c = m * 128
                for j, (a, b) in enumerate(ET_CHUNKS):
                    if a <= c < b:
                        o = et_offs[j] + kt * (b - a) + (c - a)
                        return et_all[:, o:o + 128]
                raise AssertionError(m)

            # SP queue: embT chunks (heads first), emb rows A, fixup
            # columns, embT tail; ACT queue: w halves then emb rows B
            h1, h2 = et_offs[1], et_offs[2]
            nc.sync.dma_start(et_all[:, 0:h1], embT_h[:, 0:h1])
            nc.sync.dma_start(et_all[:, h1:h2], embT_h[:, h1:h2])
            nc.sync.dma_start(er[:, 0:8], emb_h[:, 0:8 * D])
            nc.sync.dma_start(ewlab_t[:], ewlab_h[:, :])
            nc.sync.dma_start(et_all[:, h2:KT * B], embT_h[:, h2:KT * B])
            nc.scalar.dma_start(wsb_all[:, 0:2], w_h[:, 0:2 * W])
            nc.scalar.dma_start(wsb_all[:, 2:4], w_h[:, 2 * W:4 * W])
            nc.scalar.dma_start(er[:, 8:16], emb_h[:, 8 * D:16 * D])

            ones_col = persist.tile([128, 1], F16, tag="ones")
            nc.vector.memset(ones_col[:], 1.0)
            warm_rhs = persist.tile([128, 512], F16, tag="warm_rhs")
            nc.vector.memset(warm_rhs[:], 1.0)

            # preload the ACT sqrt table while the queues stream inputs
            tbl_t = persist.tile([1, 1], F32, tag="tbl")
            nc.vector.memset(tbl_t[:], 1.0)
            tbl_o = persist.tile([1, 1], F32, tag="tbl_o")
            nc.scalar.sqrt(tbl_o[:], tbl_t[:])

            # ------------ PE warm-up: hold the clock up -------------------
            warm_ps = psum.tile([1, 512], F32, tag="nps", name="warm_ps")
            order_pin = None
            for i in range(NWARM):
                order_pin = nc.tensor.matmul(
                    out=warm_ps[:, :], lhsT=ones_col[:, :], rhs=warm_rhs[:],
                    start=True, stop=True, skip_group_check=True,
                )

            # ------------ DVE: w squares (fp16, per half as DMAs land) ----
            swp4 = scratch.tile([128, KT, W], F16, tag="swp4")
            nc.vector.tensor_tensor(out=swp4[:, 0:2], in0=wsb_all[:, 0:2],
                                    in1=wsb_all[:, 0:2],
                                    op=mybir.AluOpType.mult)
            nc.vector.tensor_tensor(out=swp4[:, 2:4], in0=wsb_all[:, 2:4],
                                    in1=wsb_all[:, 2:4],
                                    op=mybir.AluOpType.mult)
            swa = scratch.tile([128, W], F16, tag="swa")
            nc.vector.tensor_tensor(out=swa[:], in0=swp4[:, 0],
                                    in1=swp4[:, 1], op=mybir.AluOpType.add)
            swb = scratch.tile([128, W], F16, tag="swb")
            nc.vector.tensor_tensor(out=swb[:], in0=swp4[:, 2],
                                    in1=swp4[:, 3], op=mybir.AluOpType.add)
            sw = scratch.tile([128, W], F16, tag="sw")
            nc.vector.tensor_tensor(out=sw[:], in0=swa[:], in1=swb[:],
                                    op=mybir.AluOpType.add)

            # ------------ m-tile matmul emitter ---------------------------
            def emit_mms(m, rhs_tiles, after):
                psC = psum.tile([128, 1536], F32, tag="psC", name="psC_%d" % m)
                first = True
                last = None
                for kt in range(KT):
                    lhsT = et_lhsT(kt, m)
                    for (c0, cn) in NCHUNKS:
                        last = nc.tensor.matmul(
                            out=psC[:, c0:c0 + cn], lhsT=lhsT,
                            rhs=rhs_tiles[kt][:, c0:c0 + cn],
                            start=(kt == 0), stop=(kt == KT - 1),
                        )
                        if first and after is not None:
                            tile.add_dep_helper(last.ins, after.ins,
                                                sync=False,
                                                reason="stream order")
                        first = False
                return psC, last

            head_raw = [
                persist.tile([128, W], F16, tag="hraw%d" % m,
                             name="hraw%d" % m)
                for m in range(HEAD)
            ]

            # heads m0..m3
            head_pss = []
            for m in range(4):
                psC, order_pin = emit_mms(m, wsb, order_pin)
                head_pss.append(psC)
                nc.scalar.copy(out=head_raw[m][:], in_=psC[:, :W])

            # w-ssq reductions right after m1 (sw ready by then)
            nps_w = []
            for j, (c0, cn) in enumerate(NCHUNKS):
                nps = psum.tile([1, 512], F32, tag="nps", name="npsw%d" % j)
                mm = nc.tensor.matmul(
                    out=nps[:, :cn], lhsT=ones_col[:, :],
                    rhs=sw[:, c0:c0 + cn], start=True, stop=True,
                )
                tile.add_dep_helper(mm.ins, order_pin.ins, sync=False,
                                    reason="order")
                order_pin = mm
                nps_w.append((nps, c0, cn))

            # 1/||w||: DVE reciprocal from PSUM, ACT sqrt, gpsimd broadcast
            rw_row = persist.tile([1, W], F32, tag="rw_row")
            for (nps, c0, cn) in nps_w:
                nc.vector.reciprocal_approx_fast(
                    out=rw_row[:, c0:c0 + cn], in_=nps[:, :cn])

            # fixup products early on DVE (fills the broadcast wait)
            elab = ewlab_t[:, 0:4 * BSL]
            wlab = ewlab_t[:, 4 * BSL:8 * BSL]
            prod = scratch.tile([128, 4 * BSL], F16, tag="prod")
            nc.vector.tensor_tensor(out=prod[:], in0=elab, in1=wlab,
                                    op=mybir.AluOpType.mult)
            sqew = scratch.tile([128, 8 * BSL], F16, tag="sqew")
            nc.vector.tensor_tensor(out=sqew[:], in0=ewlab_t[:],
                                    in1=ewlab_t[:], op=mybir.AluOpType.mult)

            rws_row = persist.tile([1, W], F16, tag="rws_row")
            nc.scalar.sqrt(rws_row[:], rw_row[:])
            ws_bc = persist.tile([128, W], F16, tag="ws_bc")
            nc.gpsimd.partition_broadcast(ws_bc[:], rws_row[:])

            # normalized rhs tiles on DVE, kt-pipelined
            wn = [
                persist.tile([128, W], F16, tag="wn%d" % kt, name="wn%d" % kt)
                for kt in range(KT)
            ]
            for kt in range(KT):
                nc.vector.tensor_tensor(out=wn[kt][:], in0=wsb[kt][:],
                                        in1=ws_bc[:], op=mybir.AluOpType.mult)

            # ------------ e row-norm tiles --------------------------------
            sq_dump = persist.tile([128, D], F16, tag="sq_dump")
            sq_dump_a = persist.tile([128, D], F16, tag="sq_dump_a")
            essq = persist.tile([128, MT], F32, tag="essq")
            rs_tmp = persist.tile([128, MT], F32, tag="rs_tmp")
            rs_em = persist.tile([128, MT], F32, tag="rs_em")

            # heads m4..m7, with the m8..15 row-norm accums interleaved on
            # ACT between the head PSUM-release copies
            for m in range(4, HEAD):
                psC, order_pin = emit_mms(m, wsb, order_pin)
                head_pss.append(psC)
                nc.scalar.copy(out=head_raw[m][:], in_=psC[:, :W])
                for k in range(2):
                    ma = 8 + 2 * (m - 4) + k
                    nc.scalar.activation(
                        sq_dump_a[:], er[:, ma],
                        mybir.ActivationFunctionType.Square,
                        accum_out=essq[:, ma:ma + 1],
                    )
            nc.vector.reciprocal_approx_fast(
                out=rs_tmp[:, 8:16], in_=essq[:, 8:16])

            # head row-norms on DVE (erA on SP queue)
            for m in range(0, HEAD):
                nc.vector.scalar_tensor_tensor(
                    out=sq_dump[:], in0=er[:, m], scalar=1.0,
                    in1=er[:, m], op0=mybir.AluOpType.mult,
                    op1=mybir.AluOpType.mult,
                    accum_out=essq[:, m:m + 1],
                )
            nc.vector.reciprocal_approx_fast(
                out=rs_tmp[:, 0:HEAD], in_=essq[:, 0:HEAD])

            def emit_rs_sqrt(m0, m1):
                # rs = S/sqrt(ssq) = sqrt(S^2 / ssq)
                nc.scalar.activation(
                    rs_em[:, m0:m1], rs_tmp[:, m0:m1],
                    mybir.ActivationFunctionType.Sqrt, scale=S * S,
                )

            # ------------ PE mains + ACT epilogue + out DMAs --------------
            # main outputs as pairs, alternating DMA queues so the writes
            # start early and spread across both queues
            ot_pairs = {}

            def emit_epilogue(m, psC):
                pr, mloc = divmod(m, 2)
                if mloc == 0:
                    ot_pairs[pr] = outp.tile([128, 2, W], F16, tag="ot",
                                             name="ot%d" % pr)
                nc.scalar.mul(ot_pairs[pr][:, mloc], psC[:, :W],
                              rs_em[:, m:m + 1])
                if mloc == 1:
                    eng = nc.scalar if pr % 2 == 1 else nc.sync
                    eng.dma_start(out_h[:, 2 * pr * W:(2 * pr + 2) * W],
                                  ot_pairs[pr][:])

            # all three fixup reductions share ONE psum bank at partition
            # offsets 0/32/64 so no pool-slot rotation can block the PE
            fix_ps = {}
            fix_ps3 = psum.tile([65, 512], F32, tag="nps", name="ps_fix")

            def emit_fix_mms(after):
                last = after
                for r, (name, src) in enumerate((
                    ("dot", prod[:, 0:4 * BSL]),
                    ("esl", sqew[:, 0:4 * BSL]),
                    ("wsl", sqew[:, 4 * BSL:8 * BSL]),
                )):
                    ps = fix_ps3[32 * r:32 * r + 1, :]
                    mm = nc.tensor.matmul(
                        out=ps, lhsT=ones_col[:, :],
                        rhs=src[:, 0:512], start=True, stop=False)
                    tile.add_dep_helper(mm.ins, last.ins, sync=False,
                                        reason="order")
                    last = nc.tensor.matmul(
                        out=ps, lhsT=ones_col[:, :],
                        rhs=src[:, 512:1024], start=False, stop=True)
                    fix_ps[name] = ps
                return last

            head_ots = {}

            def emit_head_finish(pr, eng):
                # DVE STT applies both norm scales, then DMA the pair out
                t = outp.tile([128, 2, W], F16, tag="hot", name="hot%d" % pr)
                head_ots[pr] = t
                for k in range(2):
                    m = 2 * pr + k
                    nc.vector.scalar_tensor_tensor(
                        out=t[:, k], in0=head_raw[m][:],
                        scalar=rs_em[:, m:m + 1], in1=ws_bc[:],
                        op0=mybir.AluOpType.mult, op1=mybir.AluOpType.mult,
                    )
                eng.dma_start(out_h[:, 2 * pr * W:(2 * pr + 2) * W], t[:])

            for m in range(HEAD, MT):
                if m == 8:
                    emit_rs_sqrt(8, 16)   # ACT: before epi m8
                if m == 10:
                    emit_rs_sqrt(0, 8)    # ACT: after epis m8/m9
                    emit_head_finish(0, nc.sync)
                if m == 11:
                    emit_head_finish(1, nc.sync)
                if m == 12:
                    emit_head_finish(2, nc.sync)
                if m == 13:
                    emit_head_finish(3, nc.sync)
                pss, order_pin = emit_mms(m, wn, order_pin)
                emit_epilogue(m, pss)
            # fixup reductions at the PE stream tail (PE is done anyway)
            order_pin = emit_fix_mms(order_pin)

            # ------------ fixup margin math on [1, BSL] (DVE tail) --------
            def half_add(name, ps, dt=F32):
                h0 = micro.tile([1, BSL], dt, tag="fx_h_" + name,
                                name=name + "_h0")
                nc.vector.tensor_copy(out=h0[:], in_=ps[:, 0:BSL])
                t = micro.tile([1, BSL], dt, tag="fx_" + name, name=name)
                nc.vector.tensor_tensor(out=t[:], in0=h0[:],
                                        in1=ps[:, BSL:2 * BSL],
                                        op=mybir.AluOpType.add)
                return t

            dot = half_add("dot", fix_ps["dot"])
            esl = half_add("esl", fix_ps["esl"])
            wsl = half_add("wsl", fix_ps["wsl"])

            sp_t = micro.tile([1, BSL], F32, tag="fx_sp")
            nc.vector.tensor_tensor(out=sp_t[:], in0=esl[:], in1=wsl[:],
                                    op=mybir.AluOpType.mult)
            rp = micro.tile([1, BSL], F32, tag="fx_rp")
            nc.vector.reciprocal_approx_fast(out=rp[:], in_=sp_t[:])
            rnorm = micro.tile([1, BSL], F32, tag="fx_rn")
            nc.scalar.sqrt(rnorm[:], rp[:])
            g = micro.tile([1, BSL], F32, tag="fx_g")
            nc.vector.scalar_tensor_tensor(
                out=g[:], in0=dot[:], scalar=S, in1=rnorm[:],
                op0=mybir.AluOpType.mult, op1=mybir.AluOpType.mult,
            )
            om = micro.tile([1, BSL], F32, tag="fx_om")
            nc.vector.scalar_tensor_tensor(
                out=om[:], in0=g[:], scalar=-1.0 / (S * S), in1=g[:],
                op0=mybir.AluOpType.mult, op1=mybir.AluOpType.mult,
            )
            nc.vector.tensor_scalar_add(om[:], om[:], 1.0)
            nc.vector.tensor_scalar_max(om[:], om[:], 0.0)
            sin_t = micro.tile([1, BSL], F32, tag="fx_sin")
            nc.scalar.sqrt(sin_t[:], om[:])
            cosmt = micro.tile([1, BSL], F32, tag="fx_cosmt")
            nc.vector.tensor_scalar_mul(cosmt[:], g[:], COS_M)
            nc.vector.scalar_tensor_tensor(
                out=cosmt[:], in0=sin_t[:], scalar=-S * SIN_M, in1=cosmt[:],
                op0=mybir.AluOpType.mult, op1=mybir.AluOpType.add,
            )
            keep = micro.tile([1, BSL], F32, tag="fx_keep")
            nc.vector.tensor_scalar_add(keep[:], g[:], -S * MM)
            mask = micro.tile([1, BSL], mybir.dt.uint8, tag="fx_mask")
            nc.vector.tensor_scalar(
                out=mask[:], in0=g[:], scalar1=S * THRESHOLD, scalar2=None,
                op0=mybir.AluOpType.is_gt,
            )
            val = micro.tile([1, BSL], F32, tag="fx_val")
            nc.vector.select(val[:], mask[:], cosmt[:], keep[:])
            nc.sync.dma_start(fixv_h[None, :], val[:])

    nc.finalize()
    return nc


_NC_CACHE: bass.Bass | None = None


def get_nc() -> bass.Bass:
    global _NC_CACHE
    if _NC_CACHE is None:
        _NC_CACHE = build_nc()
    return _NC_CACHE


def make_in_maps(embedding: np.ndarray, kernel: np.ndarray, labels: np.ndarray):
    embedding = np.asarray(embedding, dtype=np.float32)
    kernel = np.asarray(kernel, dtype=np.float32)
    labels = np.asarray(labels, dtype=np.int64)

    emb16 = embedding.astype(np.float16)
    embT = np.ascontiguousarray(emb16.T)
    kern_pad = np.ones((D, NCORES * SHARD), dtype=np.float32)
    kern_pad[:, :C] = kernel
    kern16 = kern_pad.astype(np.float16)

    # device layouts: partition-major, chunk-contiguous (see build_nc)
    embT4 = embT.reshape(KT, 128, B)
    et_chunks = [(0, 256), (256, 1024), (1024, 1536), (1536, 2048)]
    embT_dev = np.concatenate(
        [embT4[:, :, a:b].transpose(1, 0, 2).reshape(128, -1)
         for (a, b) in et_chunks], axis=1)
    embT_dev = np.ascontiguousarray(embT_dev)
    import ml_dtypes
    er_dev = np.ascontiguousarray(
        embedding.reshape(MT, 128, D).transpose(1, 0, 2).reshape(
            128, MT * D)).astype(ml_dtypes.float8_e4m3fn)

    in_maps = []
    for i in range(NCORES):
        wi = kern16[:, i * SHARD:(i + 1) * SHARD]
        w4 = wi.reshape(KT, 128, W)
        w_dev = np.ascontiguousarray(
            w4.transpose(1, 0, 2).reshape(128, KT * W))
        sl = slice(i * BSL, (i + 1) * BSL)
        elab = embT[:, sl].reshape(KT, 128, BSL).transpose(1, 0, 2)
        wlab = kern16[:, labels[sl]].reshape(KT, 128, BSL).transpose(1, 0, 2)
        ew = np.concatenate(
            [elab.reshape(128, KT * BSL), wlab.reshape(128, KT * BSL)], axis=1
        )
        in_maps.append(
            {
                "w": w_dev,
                "embT": embT_dev,
                "emb": er_dev,
                "ewlab": np.ascontiguousarray(ew),
            }
        )
    return in_maps


def assemble(results, labels) -> np.ndarray:
    parts = [
        np.asarray(results[i]["out"]).reshape(128, MT, W)
        .transpose(1, 0, 2).reshape(B, W)
        for i in range(NCORES)
    ]
    full = np.concatenate(parts, axis=1)[:, :C].astype(np.float32)
    fixv = np.concatenate(
        [np.asarray(results[i]["fixv"]).reshape(BSL) for i in range(NCORES)]
    ).astype(np.float32)
    labels = np.asarray(labels, dtype=np.int64)
    b = np.arange(B)
    # guard: valid margin values are bounded; fall back to the plain logit
    ok = np.isfinite(fixv) & (np.abs(fixv) < 2.0 * S)
    vals = np.where(ok, fixv, full[b, labels])
    full[b, labels] = vals
    return full


def kernel(embedding: np.ndarray, kernel: np.ndarray, labels: np.ndarray) -> np.ndarray:
    nc = get_nc()
    in_maps = make_in_maps(embedding, kernel, labels)
    last_err = None
    for _attempt in range(3):
        try:
            res = run_bass_kernel_spmd(nc, in_maps, core_ids=list(range(NCORES)))
            return assemble(res.results, labels)
        except Exception as e:  # transient NRT/device errors: retry
            last_err = e
    raise last_err


if __name__ == "__main__":
    rng = np.random.default_rng(0)
    emb = rng.standard_normal((B, D), dtype=np.float32)
    kern = (rng.standard_normal((D, C), dtype=np.float32) * 0.05).astype(np.float32)
    labs = rng.integers(0, C, size=(B,), dtype=np.int32)
    out = kernel(emb, kern, labs)
    print(out.shape, out.dtype)
